# revision 11
# baseline (speedup 1.0000x reference)
"""Trainium2 Bass kernel for batched dot-product attention.

Problem: nn_DotProductAttention (B=8, Lq=Lk=2048, D=512, fp32).
Returns (context [B,Lq,D], attn [B,Lq,Lk]) like the reference.

Sharding: batch dim across the 8 NeuronCores (1 batch element per core).

Per-core algorithm (matmuls in fp16 with fp32 PSUM accumulation; fp32
matmuls are 4x slower on the PE array, and fp16 beats bf16 on mantissa
for ~N(0,1) data at identical speed):
  1. Q,K,V load fp32 over HWDGE (in-order, so chunk completions stagger
     and the pipeline starts after the first ~1MB chunk -- SWDGE DMAs
     round-robin at packet level and would all finish together). Q and K
     transpose on the tensor engine (fp32 in, fp16 PSUM out, 128x128
     tiles against an identity) into QT/KT [d(part), L] fp16; V casts
     fp32->fp16 on the DVE. Emission interleaves transposes with the
     first score matmuls so the PE queue never stalls.
  2. Compute S_T[k,q] = sum_d K[k,d] Q[q,d] on the tensor engine
     (lhsT=KT chunk, rhs=QT block), q-half-major: once the first q-half
     of all k-tiles is done, the attn/context work for q-tiles 0-7 is
     fully unblocked and its emission is interleaved with the second
     q-half's matmuls so the PE never idles across phase boundaries
     (idle >3.4us would drop the HAM clock gate to half speed).
     The additive mask and 1/sqrt(d) scale fold into the scalar-engine
     exp: with k on partitions, bias=(mask[k]-1)*1e4/sqrt(d) is a
     per-partition activation bias. No max-subtraction is needed:
     scores are ~N(0,1) after scaling, and masked entries underflow
     exp() to exactly 0 -- same as the reference softmax.
  3. E_T (fp16) is directly the lhsT for context = attn @ V
     (contraction over k).
  4. The attn output needs the [q,k] orientation: E_T stripes go to a
     DRAM scratch (SWDGE stores, so the exp engine's queue is never
     blocked), read back per q-tile with xbar DMA-transpose on the SP
     HWDGE ring. Row sums come from a DVE reduce over those rows
     (masked entries are already exactly 0), so the attn path never
     waits on the context matmuls. Both outputs are stored fp16 and
     upcast to fp32 on the host (exact widening).
"""

import numpy as np

B = 8
LQ = 2048
LK = 2048
D = 512
P = 128
N_CORES = 8
SD = float(np.sqrt(np.float32(D)))

_NC_CACHE = {}


def _build_nc():
    import concourse.mybir as mybir
    import concourse.tile as tile
    from concourse import bacc
    from concourse.masks import make_identity

    f32 = mybir.dt.float32
    f16 = mybir.dt.float16

    K_TILES = LK // P  # 16
    Q_TILES = LQ // P  # 16
    DC = D // P  # 4 contraction chunks
    CH = 512  # staging chunk rows
    NCH = LQ // CH  # 4 chunks per tensor

    nc = bacc.Bacc(
        "TRN2", target_bir_lowering=False, debug=False, num_devices=N_CORES
    )
    q_in = nc.dram_tensor("query", [LQ, D], f32, kind="ExternalInput")
    k_in = nc.dram_tensor("key", [LK, D], f32, kind="ExternalInput")
    v_in = nc.dram_tensor("value", [LK, D], f32, kind="ExternalInput")
    m_in = nc.dram_tensor("mask", [LK], f32, kind="ExternalInput")
    attn_out = nc.dram_tensor("attn", [LQ, LK], f16, kind="ExternalOutput")
    ctx_out = nc.dram_tensor("context", [LQ, D], f16, kind="ExternalOutput")

    with tile.TileContext(nc) as tc:
        with (
            tc.tile_pool(name="dram", bufs=1, space="DRAM") as dram_pool,
            tc.tile_pool(name="big", bufs=1) as big,
            tc.tile_pool(name="small", bufs=1) as small,
            tc.tile_pool(name="stage", bufs=2) as stage_pool,
            tc.tile_pool(name="st_psum", bufs=2, space="PSUM") as st_pool,
            tc.tile_pool(name="eq", bufs=4) as eq_pool,
            tc.tile_pool(name="attn_sb", bufs=4) as attn_pool,
            tc.tile_pool(name="ctx_sb", bufs=2) as ctx_sb_pool,
            tc.tile_pool(name="rs_sb", bufs=3) as rs_pool,
        ):
            scratch_e = dram_pool.tile([LK, LQ], f16)

            # mask[p, kt] = mask_in[kt*P+p]
            mask_sb = small.tile([P, K_TILES], f32)
            nc.sync.dma_start(
                out=mask_sb[:], in_=m_in.ap().rearrange("(kt p) -> p kt", p=P)
            )
            # exp bias: (mask-1)*1e4/sqrt(d), per k partition
            bias_sb = small.tile([P, K_TILES], f32)
            nc.vector.tensor_scalar(
                out=bias_sb[:],
                in0=mask_sb[:],
                scalar1=1.0,
                scalar2=10000.0 / SD,
                op0=mybir.AluOpType.subtract,
                op1=mybir.AluOpType.mult,
            )

            identity = small.tile([P, P], f32)
            make_identity(nc, identity[:])

            QT = big.tile([P, DC, LQ], f16)  # QT[p, c, q] = Q[q, c*P+p]
            KT = big.tile([P, DC, LK], f16)
            ET = big.tile([P, K_TILES, LQ], f16)  # ET[p, kt, q] = E[q, kt*P+p]
            V = big.tile([P, K_TILES, D], f16)
            recip = small.tile([P, Q_TILES], f32)  # 1/rowsum, [q_local, qt]

            # ---- prep helpers ------------------------------------------
            transp_cm = tc.tile_pool(name="tp_psum", bufs=4, space="PSUM")
            transp_pool = transp_cm.__enter__()

            def load_transpose_chunk(src, dst_big, ch):
                """Load one 512-row fp32 chunk over HWDGE and transpose it
                into dst_big[d(part), rows] fp16 via the PE."""
                rows = slice(ch * CH, (ch + 1) * CH)
                stage = stage_pool.tile([P, CH // P, D], f32, tag="stage")
                nc.sync.dma_start(
                    out=stage[:],
                    in_=src.ap()[rows, :].rearrange("(r p) d -> p r d", p=P),
                )
                for r in range(CH // P):
                    for c in range(DC):
                        pt = transp_pool.tile([P, P], f32)
                        nc.tensor.transpose(
                            pt[:], stage[:, r, c * P : (c + 1) * P], identity[:]
                        )
                        q0 = ch * CH + r * P
                        nc.vector.tensor_copy(
                            out=dst_big[:, c, q0 : q0 + P], in_=pt[:]
                        )

            def load_v_chunk(ch):
                rows = slice(ch * CH, (ch + 1) * CH)
                stage = stage_pool.tile([P, CH // P, D], f32, tag="stage")
                nc.sync.dma_start(
                    out=stage[:],
                    in_=v_in.ap()[rows, :].rearrange("(r p) d -> p r d", p=P),
                )
                nc.vector.tensor_copy(
                    out=V[:, ch * (CH // P) : (ch + 1) * (CH // P), :],
                    in_=stage[:],
                )

            def st_block(kt, qh):
                """S_T matmuls + fused exp + scratch store for one
                (k-tile, q-half)."""
                st = st_pool.tile([P, 1024], f32)
                for qb in range(2):
                    q0 = qh * 1024 + qb * 512
                    for dc in range(DC):
                        nc.tensor.matmul(
                            st[:, qb * 512 : (qb + 1) * 512],
                            lhsT=KT[:, dc, kt * P : (kt + 1) * P],
                            rhs=QT[:, dc, q0 : q0 + 512],
                            start=(dc == 0),
                            stop=(dc == DC - 1),
                        )
                # E_T = exp(S_T/sqrt(d) + (mask-1)*1e4/sqrt(d))
                qcols = slice(qh * 1024, (qh + 1) * 1024)
                nc.scalar.activation(
                    out=ET[:, kt, qcols],
                    in_=st[:],
                    func=mybir.ActivationFunctionType.Exp,
                    bias=bias_sb[:, kt : kt + 1],
                    scale=1.0 / SD,
                )
                nc.gpsimd.dma_start(
                    out=scratch_e[kt * P : (kt + 1) * P, qcols],
                    in_=ET[:, kt, qcols],
                )

            # ---- prep + phase 1 (q-half 0), emission-interleaved --------
            load_transpose_chunk(k_in, KT, 0)
            load_transpose_chunk(q_in, QT, 0)
            load_transpose_chunk(q_in, QT, 1)
            for kt in range(4):
                st_block(kt, 0)
            for kch in range(1, NCH):
                load_transpose_chunk(k_in, KT, kch)
                for kt in range(4 * kch, 4 * kch + 4):
                    st_block(kt, 0)
            load_transpose_chunk(q_in, QT, 2)
            load_transpose_chunk(q_in, QT, 3)
            transp_cm.__exit__(None, None, None)
            for ch in range(NCH):
                load_v_chunk(ch)

            ctx_cm = tc.tile_pool(name="ctx_psum", bufs=4, space="PSUM")
            ctx_pool = ctx_cm.__enter__()

            def attn_ctx_block(qt):
                """eq transpose -> copy+rowsum (ACT) -> normalize in place
                -> attn store, plus context matmuls + store, for one
                q-tile."""
                eq = eq_pool.tile([P, LK], f16)
                nc.sync.dma_start_transpose(
                    eq[:], scratch_e[:, qt * P : (qt + 1) * P]
                )
                rs = rs_pool.tile([P, 1], f32)
                attn_sb = attn_pool.tile([P, LK], f16)
                # copy on the (otherwise idle) scalar engine; its accum_out
                # side-band produces the row sum for free
                nc.scalar.activation(
                    out=attn_sb[:],
                    in_=eq[:],
                    func=mybir.ActivationFunctionType.Copy,
                    accum_out=rs[:],
                )
                nc.vector.reciprocal(out=recip[:, qt : qt + 1], in_=rs[:])
                nc.vector.tensor_scalar_mul(
                    out=attn_sb[:], in0=attn_sb[:], scalar1=recip[:, qt : qt + 1]
                )
                nc.gpsimd.dma_start(
                    out=attn_out.ap()[qt * P : (qt + 1) * P, :], in_=attn_sb[:]
                )

                ctxp = ctx_pool.tile([P, D], f32)
                for kt in range(K_TILES):
                    nc.tensor.matmul(
                        ctxp[:],
                        lhsT=ET[:, kt, qt * P : (qt + 1) * P],
                        rhs=V[:, kt, :],
                        start=(kt == 0),
                        stop=(kt == K_TILES - 1),
                    )
                ctxs = ctx_sb_pool.tile([P, D], f16)
                nc.vector.tensor_scalar_mul(
                    out=ctxs[:], in0=ctxp[:], scalar1=recip[:, qt : qt + 1]
                )
                nc.gpsimd.dma_start(
                    out=ctx_out.ap()[qt * P : (qt + 1) * P, :], in_=ctxs[:]
                )

            # ---- phase 1 (q-half 1) interleaved with attn/context -------
            for kt in range(K_TILES):
                st_block(kt, 1)
                if kt % 2 == 1:
                    attn_ctx_block(kt // 2)
            for qt in range(8, Q_TILES):
                attn_ctx_block(qt)

            ctx_cm.__exit__(None, None, None)

    nc.finalize()
    return nc


def _get_nc():
    if "nc" not in _NC_CACHE:
        _NC_CACHE["nc"] = _build_nc()
    return _NC_CACHE["nc"]


def kernel(**inputs) -> tuple:
    from concourse.bass_utils import run_bass_kernel_spmd

    query = np.ascontiguousarray(np.asarray(inputs["query"], dtype=np.float32))
    key = np.ascontiguousarray(np.asarray(inputs["key"], dtype=np.float32))
    value = np.ascontiguousarray(np.asarray(inputs["value"], dtype=np.float32))
    mask = np.ascontiguousarray(
        np.asarray(inputs["value_attention_mask"], dtype=np.float32)
    )

    nc = _get_nc()
    in_maps = [
        {
            "query": query[b],
            "key": key[b],
            "value": value[b],
            "mask": mask[b],
        }
        for b in range(B)
    ]
    res = run_bass_kernel_spmd(nc, in_maps, core_ids=list(range(N_CORES)))
    context = np.stack([res.results[b]["context"] for b in range(B)]).astype(
        np.float32
    )
    attn = np.stack([res.results[b]["attn"] for b in range(B)]).astype(np.float32)
    return context, attn


# revision 12
# speedup vs baseline: 1.1204x; 1.1204x over previous
"""Trainium2 Bass kernel for batched dot-product attention.

Problem: nn_DotProductAttention (B=8, Lq=Lk=2048, D=512, fp32).
Returns (context [B,Lq,D], attn [B,Lq,Lk]) like the reference.

Sharding: batch dim across the 8 NeuronCores (1 batch element per core).

Per-core algorithm (matmuls in fp16 with fp32 PSUM accumulation; fp32
matmuls are 4x slower on the PE array, and fp16 beats bf16 on mantissa
for ~N(0,1) data at identical speed):
  1. Q,K,V load fp32 over HWDGE (in-order, so chunk completions stagger
     and the pipeline starts after the first ~1MB chunk -- SWDGE DMAs
     round-robin at packet level and would all finish together). Q and K
     transpose on the tensor engine (fp32 in, fp16 PSUM out, 128x128
     tiles against an identity) into QT/KT [d(part), L] fp16; V casts
     fp32->fp16 on the DVE. Emission interleaves transposes with the
     first score matmuls so the PE queue never stalls.
  2. Compute S_T[k,q] = sum_d K[k,d] Q[q,d] on the tensor engine
     (lhsT=KT chunk, rhs=QT block), q-half-major: once the first q-half
     of all k-tiles is done, the attn/context work for q-tiles 0-7 is
     fully unblocked and its emission is interleaved with the second
     q-half's matmuls so the PE never idles across phase boundaries
     (idle >3.4us would drop the HAM clock gate to half speed).
     The additive mask and 1/sqrt(d) scale fold into the scalar-engine
     exp: with k on partitions, bias=(mask[k]-1)*1e4/sqrt(d) is a
     per-partition activation bias. No max-subtraction is needed:
     scores are ~N(0,1) after scaling, and masked entries underflow
     exp() to exactly 0 -- same as the reference softmax.
  3. E_T (fp16) is directly the lhsT for context = attn @ V
     (contraction over k).
  4. The attn output needs the [q,k] orientation: E_T stripes go to a
     DRAM scratch (SWDGE stores, so the exp engine's queue is never
     blocked), read back per q-tile with xbar DMA-transpose on the SP
     HWDGE ring. Row sums come from a DVE reduce over those rows
     (masked entries are already exactly 0), so the attn path never
     waits on the context matmuls. Both outputs are stored fp16 and
     upcast to fp32 on the host (exact widening).
"""

import numpy as np

B = 8
LQ = 2048
LK = 2048
D = 512
P = 128
N_CORES = 8
SD = float(np.sqrt(np.float32(D)))

_NC_CACHE = {}


def _build_nc():
    import concourse.mybir as mybir
    import concourse.tile as tile
    from concourse import bacc
    from concourse.masks import make_identity

    f32 = mybir.dt.float32
    f16 = mybir.dt.float16

    K_TILES = LK // P  # 16
    Q_TILES = LQ // P  # 16
    DC = D // P  # 4 contraction chunks
    CH = 512  # staging chunk rows
    NCH = LQ // CH  # 4 chunks per tensor

    nc = bacc.Bacc(
        "TRN2", target_bir_lowering=False, debug=False, num_devices=N_CORES
    )
    q_in = nc.dram_tensor("query", [LQ, D], f32, kind="ExternalInput")
    k_in = nc.dram_tensor("key", [LK, D], f32, kind="ExternalInput")
    v_in = nc.dram_tensor("value", [LK, D], f32, kind="ExternalInput")
    m_in = nc.dram_tensor("mask", [LK], f32, kind="ExternalInput")
    attn_out = nc.dram_tensor("attn", [LQ, LK], f16, kind="ExternalOutput")
    ctx_out = nc.dram_tensor("context", [LQ, D], f16, kind="ExternalOutput")

    with tile.TileContext(nc) as tc:
        with (
            tc.tile_pool(name="dram", bufs=1, space="DRAM") as dram_pool,
            tc.tile_pool(name="big", bufs=1) as big,
            tc.tile_pool(name="small", bufs=1) as small,
            tc.tile_pool(name="stage", bufs=2) as stage_pool,
            tc.tile_pool(name="st_psum", bufs=2, space="PSUM") as st_pool,
            tc.tile_pool(name="eq", bufs=4) as eq_pool,
            tc.tile_pool(name="attn_sb", bufs=4) as attn_pool,
            tc.tile_pool(name="ctx_sb", bufs=2) as ctx_sb_pool,
            tc.tile_pool(name="rs_sb", bufs=4) as rs_pool,
            tc.tile_pool(name="rc_sb", bufs=4) as rc_pool,
        ):
            scratch_e = dram_pool.tile([LK, LQ], f16)

            # mask[p, kt] = mask_in[kt*P+p]
            mask_sb = small.tile([P, K_TILES], f32)
            nc.sync.dma_start(
                out=mask_sb[:], in_=m_in.ap().rearrange("(kt p) -> p kt", p=P)
            )
            # exp bias: (mask-1)*1e4/sqrt(d), per k partition
            bias_sb = small.tile([P, K_TILES], f32)
            nc.vector.tensor_scalar(
                out=bias_sb[:],
                in0=mask_sb[:],
                scalar1=1.0,
                scalar2=10000.0 / SD,
                op0=mybir.AluOpType.subtract,
                op1=mybir.AluOpType.mult,
            )

            identity = small.tile([P, P], f32)
            make_identity(nc, identity[:])

            QT = big.tile([P, DC, LQ], f16)  # QT[p, c, q] = Q[q, c*P+p]
            KT = big.tile([P, DC, LK], f16)
            ET = big.tile([P, K_TILES, LQ], f16)  # ET[p, kt, q] = E[q, kt*P+p]
            V = big.tile([P, K_TILES, D], f16)

            # ---- prep helpers ------------------------------------------
            transp_cm = tc.tile_pool(name="tp_psum", bufs=4, space="PSUM")
            transp_pool = transp_cm.__enter__()

            def load_transpose_chunk(src, dst_big, ch):
                """Load one 512-row fp32 chunk over HWDGE and transpose it
                into dst_big[d(part), rows] fp16 via the PE."""
                rows = slice(ch * CH, (ch + 1) * CH)
                stage = stage_pool.tile([P, CH // P, D], f32, tag="stage")
                nc.sync.dma_start(
                    out=stage[:],
                    in_=src.ap()[rows, :].rearrange("(r p) d -> p r d", p=P),
                )
                for r in range(CH // P):
                    for c in range(DC):
                        pt = transp_pool.tile([P, P], f32)
                        nc.tensor.transpose(
                            pt[:], stage[:, r, c * P : (c + 1) * P], identity[:]
                        )
                        q0 = ch * CH + r * P
                        nc.vector.tensor_copy(
                            out=dst_big[:, c, q0 : q0 + P], in_=pt[:]
                        )

            def load_v_chunk(ch):
                rows = slice(ch * CH, (ch + 1) * CH)
                stage = stage_pool.tile([P, CH // P, D], f32, tag="stage")
                nc.sync.dma_start(
                    out=stage[:],
                    in_=v_in.ap()[rows, :].rearrange("(r p) d -> p r d", p=P),
                )
                nc.vector.tensor_copy(
                    out=V[:, ch * (CH // P) : (ch + 1) * (CH // P), :],
                    in_=stage[:],
                )

            def st_block(kt, qh):
                """S_T matmuls + fused exp + scratch store for one
                (k-tile, q-half)."""
                st = st_pool.tile([P, 1024], f32)
                for qb in range(2):
                    q0 = qh * 1024 + qb * 512
                    for dc in range(DC):
                        nc.tensor.matmul(
                            st[:, qb * 512 : (qb + 1) * 512],
                            lhsT=KT[:, dc, kt * P : (kt + 1) * P],
                            rhs=QT[:, dc, q0 : q0 + 512],
                            start=(dc == 0),
                            stop=(dc == DC - 1),
                        )
                # E_T = exp(S_T/sqrt(d) + (mask-1)*1e4/sqrt(d))
                qcols = slice(qh * 1024, (qh + 1) * 1024)
                nc.scalar.activation(
                    out=ET[:, kt, qcols],
                    in_=st[:],
                    func=mybir.ActivationFunctionType.Exp,
                    bias=bias_sb[:, kt : kt + 1],
                    scale=1.0 / SD,
                )
                nc.gpsimd.dma_start(
                    out=scratch_e[kt * P : (kt + 1) * P, qcols],
                    in_=ET[:, kt, qcols],
                )

            # ---- prep + phase 1 (q-half 0), emission-interleaved --------
            load_transpose_chunk(k_in, KT, 0)
            load_transpose_chunk(q_in, QT, 0)
            load_transpose_chunk(q_in, QT, 1)
            for kt in range(4):
                st_block(kt, 0)
            for kch in range(1, NCH):
                load_transpose_chunk(k_in, KT, kch)
                for kt in range(4 * kch, 4 * kch + 4):
                    st_block(kt, 0)
            load_transpose_chunk(q_in, QT, 2)
            load_transpose_chunk(q_in, QT, 3)
            transp_cm.__exit__(None, None, None)
            for ch in range(NCH):
                load_v_chunk(ch)

            ctx_cm = tc.tile_pool(name="ctx_psum", bufs=4, space="PSUM")
            ctx_pool = ctx_cm.__enter__()

            def attn_ctx_block(qt):
                """eq transpose -> copy+rowsum (ACT) -> normalize in place
                -> attn store, plus context matmuls + store, for one
                q-tile."""
                eq = eq_pool.tile([P, LK], f16)
                nc.sync.dma_start_transpose(
                    eq[:], scratch_e[:, qt * P : (qt + 1) * P]
                )
                rs = rs_pool.tile([P, 1], f32)
                nc.vector.reduce_sum(
                    out=rs[:], in_=eq[:], axis=mybir.AxisListType.X
                )
                rc = rc_pool.tile([P, 1], f32)
                nc.vector.reciprocal(out=rc[:], in_=rs[:])
                attn_sb = attn_pool.tile([P, LK], f16)
                nc.vector.tensor_scalar_mul(
                    out=attn_sb[:], in0=eq[:], scalar1=rc[:]
                )
                nc.gpsimd.dma_start(
                    out=attn_out.ap()[qt * P : (qt + 1) * P, :], in_=attn_sb[:]
                )

                ctxp = ctx_pool.tile([P, D], f32)
                for kt in range(K_TILES):
                    nc.tensor.matmul(
                        ctxp[:],
                        lhsT=ET[:, kt, qt * P : (qt + 1) * P],
                        rhs=V[:, kt, :],
                        start=(kt == 0),
                        stop=(kt == K_TILES - 1),
                    )
                ctxs = ctx_sb_pool.tile([P, D], f16)
                nc.vector.tensor_scalar_mul(
                    out=ctxs[:], in0=ctxp[:], scalar1=rc[:]
                )
                nc.gpsimd.dma_start(
                    out=ctx_out.ap()[qt * P : (qt + 1) * P, :], in_=ctxs[:]
                )

            # ---- phase 1 (q-half 1) interleaved with attn/context -------
            for kt in range(K_TILES):
                st_block(kt, 1)
                if kt % 2 == 1:
                    attn_ctx_block(kt // 2)
            for qt in range(8, Q_TILES):
                attn_ctx_block(qt)

            ctx_cm.__exit__(None, None, None)

    nc.finalize()
    return nc


def _get_nc():
    if "nc" not in _NC_CACHE:
        _NC_CACHE["nc"] = _build_nc()
    return _NC_CACHE["nc"]


def kernel(**inputs) -> tuple:
    from concourse.bass_utils import run_bass_kernel_spmd

    query = np.ascontiguousarray(np.asarray(inputs["query"], dtype=np.float32))
    key = np.ascontiguousarray(np.asarray(inputs["key"], dtype=np.float32))
    value = np.ascontiguousarray(np.asarray(inputs["value"], dtype=np.float32))
    mask = np.ascontiguousarray(
        np.asarray(inputs["value_attention_mask"], dtype=np.float32)
    )

    nc = _get_nc()
    in_maps = [
        {
            "query": query[b],
            "key": key[b],
            "value": value[b],
            "mask": mask[b],
        }
        for b in range(B)
    ]
    res = run_bass_kernel_spmd(nc, in_maps, core_ids=list(range(N_CORES)))
    context = np.stack([res.results[b]["context"] for b in range(B)]).astype(
        np.float32
    )
    attn = np.stack([res.results[b]["attn"] for b in range(B)]).astype(np.float32)
    return context, attn


# revision 13
# speedup vs baseline: 1.1776x; 1.0510x over previous
"""Trainium2 Bass kernel for batched dot-product attention.

Problem: nn_DotProductAttention (B=8, Lq=Lk=2048, D=512, fp32).
Returns (context [B,Lq,D], attn [B,Lq,Lk]) like the reference.

Sharding: batch dim across the 8 NeuronCores (1 batch element per core).

Per-core algorithm (matmuls in fp16 with fp32 PSUM accumulation; fp32
matmuls are 4x slower on the PE array, and fp16 beats bf16 on mantissa
for ~N(0,1) data at identical speed):
  1. Q,K,V load fp32 over HWDGE (in-order, so chunk completions stagger
     and the pipeline starts after the first ~1MB chunk -- SWDGE DMAs
     round-robin at packet level and would all finish together). Q and K
     transpose on the tensor engine (fp32 in, fp16 PSUM out, 128x128
     tiles against an identity) into QT/KT [d(part), L] fp16; V casts
     fp32->fp16 on the DVE. Emission interleaves transposes with the
     first score matmuls so the PE queue never stalls.
  2. Compute S_T[k,q] = sum_d K[k,d] Q[q,d] on the tensor engine
     (lhsT=KT chunk, rhs=QT block), q-half-major: once the first q-half
     of all k-tiles is done, the attn/context work for q-tiles 0-7 is
     fully unblocked and its emission is interleaved with the second
     q-half's matmuls so the PE never idles across phase boundaries
     (idle >3.4us would drop the HAM clock gate to half speed).
     The additive mask and 1/sqrt(d) scale fold into the scalar-engine
     exp: with k on partitions, bias=(mask[k]-1)*1e4/sqrt(d) is a
     per-partition activation bias. No max-subtraction is needed:
     scores are ~N(0,1) after scaling, and masked entries underflow
     exp() to exactly 0 -- same as the reference softmax.
  3. E_T (fp16) is directly the lhsT for context = attn @ V
     (contraction over k).
  4. The attn output needs the [q,k] orientation: E_T stripes go to a
     DRAM scratch (SWDGE stores, so the exp engine's queue is never
     blocked), read back per q-tile with xbar DMA-transpose on the SP
     HWDGE ring. Row sums come from a DVE reduce over those rows
     (masked entries are already exactly 0), so the attn path never
     waits on the context matmuls. Both outputs are stored fp16 and
     upcast to fp32 on the host (exact widening).
"""

import numpy as np

B = 8
LQ = 2048
LK = 2048
D = 512
P = 128
N_CORES = 8
SD = float(np.sqrt(np.float32(D)))

_NC_CACHE = {}


def _build_nc():
    import concourse.mybir as mybir
    import concourse.tile as tile
    from concourse import bacc
    from concourse.masks import make_identity

    f32 = mybir.dt.float32
    f16 = mybir.dt.float16

    K_TILES = LK // P  # 16
    Q_TILES = LQ // P  # 16
    DC = D // P  # 4 contraction chunks
    CH = 512  # staging chunk rows
    NCH = LQ // CH  # 4 chunks per tensor

    nc = bacc.Bacc(
        "TRN2", target_bir_lowering=False, debug=False, num_devices=N_CORES
    )
    q_in = nc.dram_tensor("query", [LQ, D], f32, kind="ExternalInput")
    k_in = nc.dram_tensor("key", [LK, D], f32, kind="ExternalInput")
    v_in = nc.dram_tensor("value", [LK, D], f32, kind="ExternalInput")
    m_in = nc.dram_tensor("mask", [LK], f32, kind="ExternalInput")
    attn_out = nc.dram_tensor("attn", [LQ, LK], f16, kind="ExternalOutput")
    ctx_out = nc.dram_tensor("context", [LQ, D], f16, kind="ExternalOutput")

    with tile.TileContext(nc) as tc:
        with (
            tc.tile_pool(name="dram", bufs=1, space="DRAM") as dram_pool,
            tc.tile_pool(name="big", bufs=1) as big,
            tc.tile_pool(name="small", bufs=1) as small,
            tc.tile_pool(name="stage", bufs=2) as stage_pool,
            tc.tile_pool(name="st_psum", bufs=2, space="PSUM") as st_pool,
            tc.tile_pool(name="eq", bufs=6) as eq_pool,
            tc.tile_pool(name="attn_sb", bufs=4) as attn_pool,
            tc.tile_pool(name="ctx_sb", bufs=2) as ctx_sb_pool,
            tc.tile_pool(name="rs_sb", bufs=4) as rs_pool,
            tc.tile_pool(name="rc_sb", bufs=4) as rc_pool,
            tc.tile_pool(name="ctx_f32", bufs=3) as ctx_f32_pool,
        ):
            scratch_e = dram_pool.tile([LK, LQ], f16)

            # mask[p, kt] = mask_in[kt*P+p]
            mask_sb = small.tile([P, K_TILES], f32)
            nc.sync.dma_start(
                out=mask_sb[:], in_=m_in.ap().rearrange("(kt p) -> p kt", p=P)
            )
            # exp bias: (mask-1)*1e4/sqrt(d), per k partition
            bias_sb = small.tile([P, K_TILES], f32)
            nc.vector.tensor_scalar(
                out=bias_sb[:],
                in0=mask_sb[:],
                scalar1=1.0,
                scalar2=10000.0 / SD,
                op0=mybir.AluOpType.subtract,
                op1=mybir.AluOpType.mult,
            )

            identity = small.tile([P, P], f32)
            make_identity(nc, identity[:])

            QT = big.tile([P, DC, LQ], f16)  # QT[p, c, q] = Q[q, c*P+p]
            KT = big.tile([P, DC, LK], f16)
            ET = big.tile([P, K_TILES, LQ], f16)  # ET[p, kt, q] = E[q, kt*P+p]
            V = big.tile([P, K_TILES, D], f16)

            # ---- prep helpers ------------------------------------------
            transp_cm = tc.tile_pool(name="tp_psum", bufs=4, space="PSUM")
            transp_pool = transp_cm.__enter__()

            def load_transpose_chunk(src, dst_big, ch):
                """Load one 512-row fp32 chunk over HWDGE and transpose it
                into dst_big[d(part), rows] fp16 via the PE."""
                rows = slice(ch * CH, (ch + 1) * CH)
                stage = stage_pool.tile([P, CH // P, D], f32, tag="stage")
                nc.sync.dma_start(
                    out=stage[:],
                    in_=src.ap()[rows, :].rearrange("(r p) d -> p r d", p=P),
                )
                for r in range(CH // P):
                    for c in range(DC):
                        pt = transp_pool.tile([P, P], f32)
                        nc.tensor.transpose(
                            pt[:], stage[:, r, c * P : (c + 1) * P], identity[:]
                        )
                        q0 = ch * CH + r * P
                        nc.vector.tensor_copy(
                            out=dst_big[:, c, q0 : q0 + P], in_=pt[:]
                        )

            def load_v_chunk(ch):
                rows = slice(ch * CH, (ch + 1) * CH)
                stage = stage_pool.tile([P, CH // P, D], f32, tag="stage")
                nc.sync.dma_start(
                    out=stage[:],
                    in_=v_in.ap()[rows, :].rearrange("(r p) d -> p r d", p=P),
                )
                nc.vector.tensor_copy(
                    out=V[:, ch * (CH // P) : (ch + 1) * (CH // P), :],
                    in_=stage[:],
                )

            def st_block(kt, qh):
                """S_T matmuls + fused exp + scratch store for one
                (k-tile, q-half)."""
                st = st_pool.tile([P, 1024], f32)
                for qb in range(2):
                    q0 = qh * 1024 + qb * 512
                    for dc in range(DC):
                        nc.tensor.matmul(
                            st[:, qb * 512 : (qb + 1) * 512],
                            lhsT=KT[:, dc, kt * P : (kt + 1) * P],
                            rhs=QT[:, dc, q0 : q0 + 512],
                            start=(dc == 0),
                            stop=(dc == DC - 1),
                        )
                # E_T = exp(S_T/sqrt(d) + (mask-1)*1e4/sqrt(d))
                qcols = slice(qh * 1024, (qh + 1) * 1024)
                nc.scalar.activation(
                    out=ET[:, kt, qcols],
                    in_=st[:],
                    func=mybir.ActivationFunctionType.Exp,
                    bias=bias_sb[:, kt : kt + 1],
                    scale=1.0 / SD,
                )
                nc.gpsimd.dma_start(
                    out=scratch_e[kt * P : (kt + 1) * P, qcols],
                    in_=ET[:, kt, qcols],
                )

            # ---- prep + phase 1 (q-half 0), emission-interleaved --------
            load_transpose_chunk(k_in, KT, 0)
            load_transpose_chunk(q_in, QT, 0)
            load_transpose_chunk(q_in, QT, 1)
            for kt in range(4):
                st_block(kt, 0)
            for kch in range(1, NCH):
                load_transpose_chunk(k_in, KT, kch)
                if kch >= 2:
                    load_transpose_chunk(q_in, QT, kch)
                for kt in range(4 * kch, 4 * kch + 4):
                    st_block(kt, 0)
            transp_cm.__exit__(None, None, None)
            for ch in range(NCH):
                load_v_chunk(ch)

            ctx_cm = tc.tile_pool(name="ctx_psum", bufs=3, space="PSUM")
            ctx_pool = ctx_cm.__enter__()

            def attn_ctx_block(qt):
                """eq transpose -> copy+rowsum (ACT) -> normalize in place
                -> attn store, plus context matmuls + store, for one
                q-tile."""
                eq = eq_pool.tile([P, LK], f16)
                nc.sync.dma_start_transpose(
                    eq[:], scratch_e[:, qt * P : (qt + 1) * P]
                )
                rs = rs_pool.tile([P, 1], f32)
                nc.vector.reduce_sum(
                    out=rs[:], in_=eq[:], axis=mybir.AxisListType.X
                )
                rc = rc_pool.tile([P, 1], f32)
                nc.vector.reciprocal(out=rc[:], in_=rs[:])
                attn_sb = attn_pool.tile([P, LK], f16)
                nc.vector.tensor_scalar_mul(
                    out=attn_sb[:], in0=eq[:], scalar1=rc[:]
                )
                nc.gpsimd.dma_start(
                    out=attn_out.ap()[qt * P : (qt + 1) * P, :], in_=attn_sb[:]
                )

                ctxp = ctx_pool.tile([P, D], f32)
                for kt in range(K_TILES):
                    nc.tensor.matmul(
                        ctxp[:],
                        lhsT=ET[:, kt, qt * P : (qt + 1) * P],
                        rhs=V[:, kt, :],
                        start=(kt == 0),
                        stop=(kt == K_TILES - 1),
                    )
                ctxf = ctx_f32_pool.tile([P, D], f32)
                nc.scalar.copy(out=ctxf[:], in_=ctxp[:])
                ctxs = ctx_sb_pool.tile([P, D], f16)
                nc.vector.tensor_scalar_mul(
                    out=ctxs[:], in0=ctxf[:], scalar1=rc[:]
                )
                nc.gpsimd.dma_start(
                    out=ctx_out.ap()[qt * P : (qt + 1) * P, :], in_=ctxs[:]
                )

            # ---- phase 1 (q-half 1) interleaved with attn/context -------
            for kt in range(K_TILES):
                st_block(kt, 1)
                if kt % 2 == 1:
                    attn_ctx_block(kt // 2)
            for qt in range(8, Q_TILES):
                attn_ctx_block(qt)

            ctx_cm.__exit__(None, None, None)

    nc.finalize()
    return nc


def _get_nc():
    if "nc" not in _NC_CACHE:
        _NC_CACHE["nc"] = _build_nc()
    return _NC_CACHE["nc"]


def kernel(**inputs) -> tuple:
    from concourse.bass_utils import run_bass_kernel_spmd

    query = np.ascontiguousarray(np.asarray(inputs["query"], dtype=np.float32))
    key = np.ascontiguousarray(np.asarray(inputs["key"], dtype=np.float32))
    value = np.ascontiguousarray(np.asarray(inputs["value"], dtype=np.float32))
    mask = np.ascontiguousarray(
        np.asarray(inputs["value_attention_mask"], dtype=np.float32)
    )

    nc = _get_nc()
    in_maps = [
        {
            "query": query[b],
            "key": key[b],
            "value": value[b],
            "mask": mask[b],
        }
        for b in range(B)
    ]
    res = run_bass_kernel_spmd(nc, in_maps, core_ids=list(range(N_CORES)))
    context = np.stack([res.results[b]["context"] for b in range(B)]).astype(
        np.float32
    )
    attn = np.stack([res.results[b]["attn"] for b in range(B)]).astype(np.float32)
    return context, attn
